# revision 24
# baseline (speedup 1.0000x reference)
"""Trainium2 Bass kernel v2 for nn_NMPN (GNN message passing), 8 NeuronCores.

Changes vs v1:
  - gathers use dma_gather (int16 idx, 768 rows/call) on a TWO-BANK table
    (rows [0,32768) / [32768,50016)); slots whose source row is in the
    other bank point at that bank's zero rows, so gA + gB merges exactly.
  - neighbor aggregation on DVE (6 fp8 merge-adds + 5 bf16 sum-adds),
    then only 4 PE transpose matmuls + 4 bf16 main matmuls per chunk.
  - tables/AllGathers in fp8e4m3 for depths 0-2, bf16 for the final
    depth's table (precision: rel_fro ~8e-3 vs 2e-2 gate).
  - AllGather outputs in "Shared" scratchpad (pair-HBM, one copy per
    HBM pair).
"""

import os
import numpy as np
import ml_dtypes

import concourse.bass as bass
import concourse.mybir as mybir
import concourse.tile as tile
from concourse import bacc
from concourse.bass_utils import run_bass_kernel_spmd
from concourse.masks import make_identity

NCORES = 8
N_ATOMS = 50000
MAX_NB = 6
ATOM_FDIM = 39
BOND_FDIM = 11
HIDDEN = 512
DEPTH = 4

A_LOC = N_ATOMS // NCORES            # 6250 atoms per core
NCHUNK = (A_LOC + 127) // 128        # 49 chunks
LAST_M = A_LOC - (NCHUNK - 1) * 128  # 106 atoms in last chunk
FB = MAX_NB * BOND_FDIM              # 66

# ---- single table, segment-major rows; zero row at the end ----
ZROW = N_ATOMS                       # zero row index
TAB_ROWS = N_ATOMS + 8

N_SEG = int(os.environ.get("TRN_NSEG", "7"))


def _mk_segs():
    # seg 0 and seg 1 are small; chunks are processed in the order
    # seg1-first, seg0-last, so the AllGather chain starts early and the
    # depth-gating final AllGather is tiny.
    taper = os.environ.get(
        "TRN_SEG_TAPER", "256,1408,2304,2026,256")
    sizes = [int(x) for x in taper.split(",")]
    assert sum(sizes) == A_LOC, sizes
    segs = []
    a = 0
    for m in sizes:
        segs.append((a, m))
        a += m
    return segs


SEGS = _mk_segs()
CHUNK_ORDER = [2, 3] + list(range(4, NCHUNK)) + [0, 1]
# table rows are assigned in AG-emission order so ranged gathers [0, L)
# exclude late-fired segments; seg 0 (processed last) gets the top rows
PROC_ORDER = [1, 2, 3, 4, 0]
def _seg_row0(si):
    r = 0
    for pj in PROC_ORDER:
        if pj == si:
            return NCORES * r
        r += SEGS[pj][1]
    raise AssertionError(si)


def _row_of(core, a):
    """table row of per-core atom index a on core `core` (emission-major)."""
    for si, (a0, m) in enumerate(SEGS):
        if a0 <= a < a0 + m:
            return _seg_row0(si) + core * m + (a - a0)
    raise AssertionError(a)


F32 = mybir.dt.float32
F32R = mybir.dt.float32r
BF16 = mybir.dt.bfloat16
FP8 = mybir.dt.float8e4
U16 = mybir.dt.uint16
I32 = mybir.dt.int32


def build_nc(L):
    nc = bacc.Bacc("TRN2", target_bir_lowering=False, num_devices=NCORES)

    fatoms_t = nc.dram_tensor("fatoms_t", [ATOM_FDIM, A_LOC], BF16, kind="ExternalInput")
    fbg_t = nc.dram_tensor("fbg_t", [FB, A_LOC], BF16, kind="ExternalInput")
    src_idx = nc.dram_tensor("src_idx", [128, NCHUNK * MAX_NB], I32, kind="ExternalInput")
    w_nin_t = nc.dram_tensor("w_nin_t", [ATOM_FDIM, HIDDEN], BF16, kind="ExternalInput")
    wb_rep = nc.dram_tensor("wb_rep", [FB, HIDDEN], BF16, kind="ExternalInput")
    w_h_t = nc.dram_tensor("w_h_t", [HIDDEN, HIDDEN], BF16, kind="ExternalInput")
    h_out = nc.dram_tensor("h_out", [A_LOC, HIDDEN], F32, kind="ExternalOutput")

    agin8 = nc.dram_tensor("agin8", [A_LOC, HIDDEN // 2], U16)
    agin16 = nc.dram_tensor("agin16", [A_LOC, HIDDEN], BF16)
    # tables: d0<-t8a (setup), d1<-t8b, d2<-t8a, d3<-t16
    t8a = nc.dram_tensor("t8a", [TAB_ROWS, HIDDEN // 2], U16, addr_space="Shared")
    t8b = nc.dram_tensor("t8b", [TAB_ROWS, HIDDEN // 2], U16, addr_space="Shared")
    t16 = nc.dram_tensor("t16", [TAB_ROWS, HIDDEN], BF16, addr_space="Shared")
    seed_dram = nc.dram_tensor("seed_dram", [1, 16], F32)

    rg = [list(range(NCORES))]

    # segment AG bookkeeping: for seg (a0, m) find trigger chunk + row base
    def seg_plan():
        pos = {c: i for i, c in enumerate(CHUNK_ORDER)}
        plan = []
        for si, (a0, m) in enumerate(SEGS):
            chunks = range(a0 // 128, (a0 + m + 127) // 128)
            trig = max(chunks, key=lambda c: pos[c])
            row0 = _seg_row0(si)
            plan.append((trig, a0, m, row0))
        return plan

    PLAN = seg_plan()

    def emit_ags(nc, agin, t_out, c):
        cc = None
        for (trig, a0, m, row0) in PLAN:
            if c == trig:
                cc = nc.gpsimd.collective_compute(
                    "AllGather", mybir.AluOpType.bypass,
                    replica_groups=rg,
                    ins=[agin[a0:a0 + m, :]],
                    outs=[t_out[row0:row0 + NCORES * m, :]],
                ) or cc
        return cc

    with tile.TileContext(nc) as tc:
        with (
            tc.tile_pool(name="persist", bufs=1) as pp,
            tc.tile_pool(name="psum", bufs=4, space="PSUM") as psp,
            tc.tile_pool(name="work", bufs=6) as wp,
            tc.tile_pool(name="out", bufs=4) as op,
        ):
            base_t = pp.tile([128, NCHUNK * HIDDEN], BF16, tag="base")
            ident = pp.tile([128, 128], BF16, tag="ident")
            make_identity(nc, ident[:, :])
            src_sb = pp.tile([128, NCHUNK * MAX_NB], I32, tag="src")
            la = nc.sync.dma_start(out=src_sb[:, :], in_=src_idx[:, :])
            whs = pp.tile([128, 4 * HIDDEN], BF16, tag="wh")
            for b in range(4):
                nc.sync.dma_start(
                    out=whs[:, b * HIDDEN:(b + 1) * HIDDEN],
                    in_=w_h_t[b * 128:(b + 1) * 128, :],
                )
            zeros8 = pp.tile([8, HIDDEN // 2], U16, tag="zr8")
            nc.vector.memset(zeros8[:, :], 0.0)
            zeros16 = pp.tile([8, HIDDEN], BF16, tag="zr16")
            nc.vector.memset(zeros16[:, :], 0.0)
            zeros_f = pp.tile([1, 16], F32, tag="zf")
            nc.vector.memset(zeros_f[:, :], 0.0)
            nc.sync.dma_start(out=seed_dram[:, :], in_=zeros_f[:, :])
            zrow_w = []
            for t, z in ((t8a, zeros8), (t8b, zeros8), (t16, zeros16)):
                zrow_w.append(nc.sync.dma_start(
                    out=t[ZROW:ZROW + 8, :], in_=z[:, :]))
            nc.vector.memset(base_t[:, :], 0.0)

            # ---------- setup: base = relu(fatoms@Wnin.T) + fbgather@Wbrep ----
            cc = None
            with tc.tile_pool(name="setup", bufs=6) as sp:
                wnin_sb = pp.tile([ATOM_FDIM, HIDDEN], BF16, tag="wnin")
                nc.sync.dma_start(out=wnin_sb[:, :], in_=w_nin_t[:, :])
                wbr_sb = pp.tile([FB, HIDDEN], BF16, tag="wbr")
                nc.sync.dma_start(out=wbr_sb[:, :], in_=wb_rep[:, :])

                for c in CHUNK_ORDER:
                    m = 128 if c < NCHUNK - 1 else LAST_M
                    a0 = c * 128
                    fa_sb = sp.tile([ATOM_FDIM, 128], BF16, tag="fa")
                    nc.sync.dma_start(out=fa_sb[:, :m], in_=fatoms_t[:, a0:a0 + m])
                    fbg_sb = sp.tile([FB, 128], BF16, tag="fbg")
                    nc.sync.dma_start(out=fbg_sb[:, :m], in_=fbg_t[:, a0:a0 + m])
                    ps_h0 = psp.tile([128, HIDDEN], F32, tag="ps_h0")
                    nc.tensor.matmul(
                        out=ps_h0[:m, :], lhsT=fa_sb[:, :m], rhs=wnin_sb[:, :],
                        start=True, stop=True,
                    )
                    ps_b = psp.tile([128, HIDDEN], F32, tag="ps_b")
                    nc.tensor.matmul(
                        out=ps_b[:m, :], lhsT=fbg_sb[:, :m], rhs=wbr_sb[:, :],
                        start=True, stop=True,
                    )
                    h0f = sp.tile([128, HIDDEN], F32, tag="h0f")
                    nc.scalar.activation(
                        h0f[:m, :], ps_h0[:m, :],
                        mybir.ActivationFunctionType.Relu,
                    )
                    nc.vector.tensor_add(
                        base_t[:m, c * HIDDEN:(c + 1) * HIDDEN],
                        h0f[:m, :], ps_b[:m, :],
                    )
                    h0b = sp.tile([128, HIDDEN], FP8, tag="h0b")
                    nc.vector.tensor_copy(h0b[:m, :], h0f[:m, :])
                    nc.sync.dma_start(out=agin8[a0:a0 + m, :], in_=h0b[:m, :].bitcast(U16))
                    cc = emit_ags(nc, agin8, t8a, c) or cc

            def seed(dep_insts, tag):
                prev = None
                for i, d in enumerate(dep_insts):
                    if d is None:
                        continue
                    st = wp.tile([1, 16], F32, tag=f"seed_{tag}_{i}")
                    s = nc.gpsimd.dma_start(out=st[:, :], in_=seed_dram[:, :])
                    tile.add_dep_helper(s.ins, d.ins, sync=True, reason=f"seed {tag}")
                    if prev is not None:
                        tile.add_dep_helper(s.ins, prev.ins, sync=False, reason="chain")
                    prev = s
                return prev


            # ---------- depth loop ----------
            tables = [t8a, t8b, t8a, t8b]
            for d in range(DEPTH):
                t_in = tables[d]
                last = d == DEPTH - 1
                for c in CHUNK_ORDER:
                    m = 128 if c < NCHUNK - 1 else LAST_M
                    a0 = c * 128
                    g = wp.tile([128, MAX_NB, HIDDEN // 2], U16, tag="g")
                    for j in range(MAX_NB):
                        nc.gpsimd.indirect_dma_start(
                            out=g[:, j, :],
                            out_offset=None,
                            in_=t_in[0:int(L[c, j]), :],
                            in_offset=bass.IndirectOffsetOnAxis(
                                ap=src_sb[:, c * MAX_NB + j:c * MAX_NB + j + 1],
                                axis=0,
                            ),
                        )
                    # neighbor sum on DVE
                    sa = wp.tile([128, HIDDEN], BF16, tag="sa")
                    nc.vector.tensor_add(sa[:, :], g[:, 0, :].bitcast(FP8), g[:, 1, :].bitcast(FP8))
                    sb = wp.tile([128, HIDDEN], BF16, tag="sb")
                    nc.vector.tensor_add(sb[:, :], g[:, 2, :].bitcast(FP8), g[:, 3, :].bitcast(FP8))
                    sc = wp.tile([128, HIDDEN], BF16, tag="sc")
                    nc.vector.tensor_add(sc[:, :], g[:, 4, :].bitcast(FP8), g[:, 5, :].bitcast(FP8))
                    nc.vector.tensor_add(sa[:, :], sa[:, :], sb[:, :])
                    s = wp.tile([128, HIDDEN], BF16, tag="s")
                    nc.vector.tensor_add(s[:, :], sa[:, :], sc[:, :])
                    # transpose via PE
                    ps_nt = psp.tile([128, HIDDEN], F32, tag="ps_h0")
                    for b in range(4):
                        nc.tensor.matmul(
                            out=ps_nt[:, b * 128:(b + 1) * 128],
                            lhsT=s[:, b * 128:(b + 1) * 128],
                            rhs=ident[:, :],
                            start=True, stop=True,
                        )
                    nt = wp.tile([128, HIDDEN], BF16, tag="nt")
                    nc.scalar.copy(nt[:, :], ps_nt[:, :])
                    ps_o = psp.tile([128, HIDDEN], F32, tag="ps_b")
                    for b in range(4):
                        nc.tensor.matmul(
                            out=ps_o[:, :],
                            lhsT=nt[:, b * 128:(b + 1) * 128],
                            rhs=whs[:, b * HIDDEN:(b + 1) * HIDDEN],
                            start=(b == 0), stop=(b == 3),
                        )
                    tnew = op.tile([128, HIDDEN], F32, tag="tnew")
                    nc.vector.tensor_add(
                        tnew[:, :], ps_o[:, :], base_t[:, c * HIDDEN:(c + 1) * HIDDEN]
                    )
                    if last:
                        hf = op.tile([128, HIDDEN], F32, tag="hf")
                        nc.scalar.activation(
                            hf[:, :], tnew[:, :], mybir.ActivationFunctionType.Relu
                        )
                        nc.sync.dma_start(out=h_out[a0:a0 + m, :], in_=hf[:m, :])
                    else:
                        hb = op.tile([128, HIDDEN], FP8, tag="hb")
                        nc.scalar.activation(
                            hb[:, :], tnew[:, :], mybir.ActivationFunctionType.Relu
                        )
                        nc.sync.dma_start(
                            out=agin8[a0:a0 + m, :], in_=hb[:m, :].bitcast(U16))
                        cc = emit_ags(nc, agin8, tables[d + 1], c) or cc


    nc.finalize()
    return nc


def _prepare_inputs(fatoms, fbonds, W_nin, W_node, aoutgraph, in_n):
    fatoms = np.asarray(fatoms, dtype=np.float32)
    fbonds = np.asarray(fbonds, dtype=np.float32)
    W_nin = np.asarray(W_nin, dtype=np.float32)
    W_node = np.asarray(W_node, dtype=np.float32)
    aout = np.asarray(aoutgraph, dtype=np.int64)
    in_n = np.asarray(in_n, dtype=np.int64)

    # atom id -> table row (segment-major two-bank layout)
    perm = np.empty(N_ATOMS, dtype=np.int64)
    rows_core0 = np.array([_row_of(0, a) for a in range(A_LOC)], dtype=np.int64)
    seg_of = np.empty(A_LOC, dtype=np.int64)
    seg_m = np.empty(A_LOC, dtype=np.int64)
    for i, (a0, mseg) in enumerate(SEGS):
        seg_of[a0:a0 + mseg] = i
        seg_m[a0:a0 + mseg] = mseg
    for k in range(NCORES):
        perm[k * A_LOC:(k + 1) * A_LOC] = rows_core0 + k * seg_m
    # src[a,j]: table row of source atom, or -1 for the padding bond
    src_atom = np.where(aout > 0, in_n[np.maximum(aout - 1, 0)], -1)
    src_row = np.where(src_atom >= 0, perm[np.maximum(src_atom, 0)], -1)

    w_nin_t = np.ascontiguousarray(W_nin.T).astype(ml_dtypes.bfloat16)
    w_h_t = np.ascontiguousarray(W_node[:, :HIDDEN].T).astype(ml_dtypes.bfloat16)
    wb = W_node[:, HIDDEN:]
    wb_rep = np.ascontiguousarray(np.tile(wb.T, (MAX_NB, 1))).astype(ml_dtypes.bfloat16)

    in_maps = []
    for k in range(NCORES):
        sh = slice(k * A_LOC, (k + 1) * A_LOC)
        fat = np.ascontiguousarray(fatoms[sh].T).astype(ml_dtypes.bfloat16)
        fbg = fbonds[aout[sh]].reshape(A_LOC, FB)
        fbg_t = np.ascontiguousarray(fbg.T).astype(ml_dtypes.bfloat16)
        rows_k = np.where(src_row[sh] >= 0, src_row[sh], ZROW)  # [6250, 6]
        rows_k = np.sort(rows_k, axis=1)  # ascending: call j only needs rows < L[c,j]
        arr = np.full((128, NCHUNK * MAX_NB), ZROW, dtype=np.int32)
        for c in range(NCHUNK):
            mm = 128 if c < NCHUNK - 1 else LAST_M
            arr[:mm, c * MAX_NB:(c + 1) * MAX_NB] = rows_k[c * 128:c * 128 + mm]
        in_maps.append({
            "fatoms_t": fat,
            "fbg_t": fbg_t,
            "src_idx": arr,
            "w_nin_t": w_nin_t,
            "wb_rep": wb_rep,
            "w_h_t": w_h_t,
        })
    # static per-(chunk, call) gather row limits: max over cores
    L = np.zeros((NCHUNK, MAX_NB), dtype=np.int64)
    for im in in_maps:
        a = im["src_idx"]
        for c in range(NCHUNK):
            L[c] = np.maximum(L[c], a[:, c * MAX_NB:(c + 1) * MAX_NB].max(axis=0) + 1)
    return in_maps, L


_cached_nc = None
_cached_key = None


def _get_nc(L):
    global _cached_nc, _cached_key
    key = L.tobytes()
    if _cached_nc is None or _cached_key != key:
        _cached_nc = build_nc(L)
        _cached_key = key
    return _cached_nc


def run(inputs, trace=False):
    in_maps, L = _prepare_inputs(**inputs)
    nc = _get_nc(L)
    res = run_bass_kernel_spmd(
        nc, in_maps, core_ids=list(range(NCORES)), trace=trace
    )
    h_full = np.concatenate([res.results[c]["h_out"] for c in range(NCORES)], axis=0)
    out = np.ascontiguousarray(h_full.T)
    return out, res


def kernel(**inputs) -> np.ndarray:
    trace = bool(int(os.environ.get("TRN_KERNEL_TRACE", "0")))
    out, _ = run(inputs, trace=trace)
    return out


# revision 25
# speedup vs baseline: 1.0399x; 1.0399x over previous
"""Trainium2 Bass kernel v2 for nn_NMPN (GNN message passing), 8 NeuronCores.

Changes vs v1:
  - gathers use dma_gather (int16 idx, 768 rows/call) on a TWO-BANK table
    (rows [0,32768) / [32768,50016)); slots whose source row is in the
    other bank point at that bank's zero rows, so gA + gB merges exactly.
  - neighbor aggregation on DVE (6 fp8 merge-adds + 5 bf16 sum-adds),
    then only 4 PE transpose matmuls + 4 bf16 main matmuls per chunk.
  - tables/AllGathers in fp8e4m3 for depths 0-2, bf16 for the final
    depth's table (precision: rel_fro ~8e-3 vs 2e-2 gate).
  - AllGather outputs in "Shared" scratchpad (pair-HBM, one copy per
    HBM pair).
"""

import os
import numpy as np
import ml_dtypes

import concourse.bass as bass
import concourse.mybir as mybir
import concourse.tile as tile
from concourse import bacc
from concourse.bass_utils import run_bass_kernel_spmd
from concourse.masks import make_identity

NCORES = 8
N_ATOMS = 50000
MAX_NB = 6
ATOM_FDIM = 39
BOND_FDIM = 11
HIDDEN = 512
DEPTH = 4

A_LOC = N_ATOMS // NCORES            # 6250 atoms per core
NCHUNK = (A_LOC + 127) // 128        # 49 chunks
LAST_M = A_LOC - (NCHUNK - 1) * 128  # 106 atoms in last chunk
FB = MAX_NB * BOND_FDIM              # 66

# ---- single table, segment-major rows; zero row at the end ----
ZROW = N_ATOMS                       # zero row index
TAB_ROWS = N_ATOMS + 8

N_SEG = int(os.environ.get("TRN_NSEG", "7"))


def _mk_segs():
    # seg 0 and seg 1 are small; chunks are processed in the order
    # seg1-first, seg0-last, so the AllGather chain starts early and the
    # depth-gating final AllGather is tiny.
    taper = os.environ.get(
        "TRN_SEG_TAPER", "256,256,1152,1152,1152,1152,874,256")
    sizes = [int(x) for x in taper.split(",")]
    assert sum(sizes) == A_LOC, sizes
    segs = []
    a = 0
    for m in sizes:
        segs.append((a, m))
        a += m
    return segs


SEGS = _mk_segs()
CHUNK_ORDER = [2, 3] + list(range(4, NCHUNK)) + [0, 1]
# table rows are assigned in AG-emission order so ranged gathers [0, L)
# exclude late-fired segments; seg 0 (processed last) gets the top rows
PROC_ORDER = [1, 2, 3, 4, 5, 6, 7, 0]
def _seg_row0(si):
    r = 0
    for pj in PROC_ORDER:
        if pj == si:
            return NCORES * r
        r += SEGS[pj][1]
    raise AssertionError(si)


def _row_of(core, a):
    """table row of per-core atom index a on core `core` (emission-major)."""
    for si, (a0, m) in enumerate(SEGS):
        if a0 <= a < a0 + m:
            return _seg_row0(si) + core * m + (a - a0)
    raise AssertionError(a)


F32 = mybir.dt.float32
F32R = mybir.dt.float32r
BF16 = mybir.dt.bfloat16
FP8 = mybir.dt.float8e4
U16 = mybir.dt.uint16
I32 = mybir.dt.int32


def build_nc(L):
    nc = bacc.Bacc("TRN2", target_bir_lowering=False, num_devices=NCORES)

    fatoms_t = nc.dram_tensor("fatoms_t", [ATOM_FDIM, A_LOC], BF16, kind="ExternalInput")
    fbg_t = nc.dram_tensor("fbg_t", [FB, A_LOC], BF16, kind="ExternalInput")
    src_idx = nc.dram_tensor("src_idx", [128, NCHUNK * MAX_NB], I32, kind="ExternalInput")
    w_nin_t = nc.dram_tensor("w_nin_t", [ATOM_FDIM, HIDDEN], BF16, kind="ExternalInput")
    wb_rep = nc.dram_tensor("wb_rep", [FB, HIDDEN], BF16, kind="ExternalInput")
    w_h_t = nc.dram_tensor("w_h_t", [HIDDEN, HIDDEN], BF16, kind="ExternalInput")
    h_out = nc.dram_tensor("h_out", [A_LOC, HIDDEN], F32, kind="ExternalOutput")

    agin8 = nc.dram_tensor("agin8", [A_LOC, HIDDEN // 2], U16)
    agin16 = nc.dram_tensor("agin16", [A_LOC, HIDDEN], BF16)
    # tables: d0<-t8a (setup), d1<-t8b, d2<-t8a, d3<-t16
    t8a = nc.dram_tensor("t8a", [TAB_ROWS, HIDDEN // 2], U16, addr_space="Shared")
    t8b = nc.dram_tensor("t8b", [TAB_ROWS, HIDDEN // 2], U16, addr_space="Shared")
    t16 = nc.dram_tensor("t16", [TAB_ROWS, HIDDEN], BF16, addr_space="Shared")
    seed_dram = nc.dram_tensor("seed_dram", [1, 16], F32)

    rg = [list(range(NCORES))]

    # segment AG bookkeeping: for seg (a0, m) find trigger chunk + row base
    def seg_plan():
        pos = {c: i for i, c in enumerate(CHUNK_ORDER)}
        plan = []
        for si, (a0, m) in enumerate(SEGS):
            chunks = range(a0 // 128, (a0 + m + 127) // 128)
            trig = max(chunks, key=lambda c: pos[c])
            row0 = _seg_row0(si)
            plan.append((trig, a0, m, row0))
        return plan

    PLAN = seg_plan()

    def emit_ags(nc, agin, t_out, c):
        cc = None
        for (trig, a0, m, row0) in PLAN:
            if c == trig:
                cc = nc.gpsimd.collective_compute(
                    "AllGather", mybir.AluOpType.bypass,
                    replica_groups=rg,
                    ins=[agin[a0:a0 + m, :]],
                    outs=[t_out[row0:row0 + NCORES * m, :]],
                ) or cc
        return cc

    with tile.TileContext(nc) as tc:
        with (
            tc.tile_pool(name="persist", bufs=1) as pp,
            tc.tile_pool(name="psum", bufs=4, space="PSUM") as psp,
            tc.tile_pool(name="work", bufs=6) as wp,
            tc.tile_pool(name="out", bufs=4) as op,
        ):
            base_t = pp.tile([128, NCHUNK * HIDDEN], BF16, tag="base")
            ident = pp.tile([128, 128], BF16, tag="ident")
            make_identity(nc, ident[:, :])
            src_sb = pp.tile([128, NCHUNK * MAX_NB], I32, tag="src")
            la = nc.sync.dma_start(out=src_sb[:, :], in_=src_idx[:, :])
            whs = pp.tile([128, 4 * HIDDEN], BF16, tag="wh")
            for b in range(4):
                nc.sync.dma_start(
                    out=whs[:, b * HIDDEN:(b + 1) * HIDDEN],
                    in_=w_h_t[b * 128:(b + 1) * 128, :],
                )
            zeros8 = pp.tile([8, HIDDEN // 2], U16, tag="zr8")
            nc.vector.memset(zeros8[:, :], 0.0)
            zeros16 = pp.tile([8, HIDDEN], BF16, tag="zr16")
            nc.vector.memset(zeros16[:, :], 0.0)
            zeros_f = pp.tile([1, 16], F32, tag="zf")
            nc.vector.memset(zeros_f[:, :], 0.0)
            nc.sync.dma_start(out=seed_dram[:, :], in_=zeros_f[:, :])
            zrow_w = []
            for t, z in ((t8a, zeros8), (t8b, zeros8), (t16, zeros16)):
                zrow_w.append(nc.sync.dma_start(
                    out=t[ZROW:ZROW + 8, :], in_=z[:, :]))
            nc.vector.memset(base_t[:, :], 0.0)

            # ---------- setup: base = relu(fatoms@Wnin.T) + fbgather@Wbrep ----
            cc = None
            with tc.tile_pool(name="setup", bufs=6) as sp:
                wnin_sb = pp.tile([ATOM_FDIM, HIDDEN], BF16, tag="wnin")
                nc.sync.dma_start(out=wnin_sb[:, :], in_=w_nin_t[:, :])
                wbr_sb = pp.tile([FB, HIDDEN], BF16, tag="wbr")
                nc.sync.dma_start(out=wbr_sb[:, :], in_=wb_rep[:, :])

                for c in CHUNK_ORDER:
                    m = 128 if c < NCHUNK - 1 else LAST_M
                    a0 = c * 128
                    fa_sb = sp.tile([ATOM_FDIM, 128], BF16, tag="fa")
                    nc.sync.dma_start(out=fa_sb[:, :m], in_=fatoms_t[:, a0:a0 + m])
                    fbg_sb = sp.tile([FB, 128], BF16, tag="fbg")
                    nc.sync.dma_start(out=fbg_sb[:, :m], in_=fbg_t[:, a0:a0 + m])
                    ps_h0 = psp.tile([128, HIDDEN], F32, tag="ps_h0")
                    nc.tensor.matmul(
                        out=ps_h0[:m, :], lhsT=fa_sb[:, :m], rhs=wnin_sb[:, :],
                        start=True, stop=True,
                    )
                    ps_b = psp.tile([128, HIDDEN], F32, tag="ps_b")
                    nc.tensor.matmul(
                        out=ps_b[:m, :], lhsT=fbg_sb[:, :m], rhs=wbr_sb[:, :],
                        start=True, stop=True,
                    )
                    h0f = sp.tile([128, HIDDEN], F32, tag="h0f")
                    nc.scalar.activation(
                        h0f[:m, :], ps_h0[:m, :],
                        mybir.ActivationFunctionType.Relu,
                    )
                    nc.vector.tensor_add(
                        base_t[:m, c * HIDDEN:(c + 1) * HIDDEN],
                        h0f[:m, :], ps_b[:m, :],
                    )
                    h0b = sp.tile([128, HIDDEN], FP8, tag="h0b")
                    nc.vector.tensor_copy(h0b[:m, :], h0f[:m, :])
                    nc.sync.dma_start(out=agin8[a0:a0 + m, :], in_=h0b[:m, :].bitcast(U16))
                    cc = emit_ags(nc, agin8, t8a, c) or cc

            def seed(dep_insts, tag):
                prev = None
                for i, d in enumerate(dep_insts):
                    if d is None:
                        continue
                    st = wp.tile([1, 16], F32, tag=f"seed_{tag}_{i}")
                    s = nc.gpsimd.dma_start(out=st[:, :], in_=seed_dram[:, :])
                    tile.add_dep_helper(s.ins, d.ins, sync=True, reason=f"seed {tag}")
                    if prev is not None:
                        tile.add_dep_helper(s.ins, prev.ins, sync=False, reason="chain")
                    prev = s
                return prev


            # ---------- depth loop ----------
            tables = [t8a, t8b, t8a, t8b]
            for d in range(DEPTH):
                t_in = tables[d]
                last = d == DEPTH - 1
                for c in CHUNK_ORDER:
                    m = 128 if c < NCHUNK - 1 else LAST_M
                    a0 = c * 128
                    g = wp.tile([128, MAX_NB, HIDDEN // 2], U16, tag="g")
                    for j in range(MAX_NB):
                        nc.gpsimd.indirect_dma_start(
                            out=g[:, j, :],
                            out_offset=None,
                            in_=t_in[0:int(L[c, j]), :],
                            in_offset=bass.IndirectOffsetOnAxis(
                                ap=src_sb[:, c * MAX_NB + j:c * MAX_NB + j + 1],
                                axis=0,
                            ),
                        )
                    # neighbor sum on DVE
                    sa = wp.tile([128, HIDDEN], BF16, tag="sa")
                    nc.vector.tensor_add(sa[:, :], g[:, 0, :].bitcast(FP8), g[:, 1, :].bitcast(FP8))
                    sb = wp.tile([128, HIDDEN], BF16, tag="sb")
                    nc.vector.tensor_add(sb[:, :], g[:, 2, :].bitcast(FP8), g[:, 3, :].bitcast(FP8))
                    sc = wp.tile([128, HIDDEN], BF16, tag="sc")
                    nc.vector.tensor_add(sc[:, :], g[:, 4, :].bitcast(FP8), g[:, 5, :].bitcast(FP8))
                    nc.vector.tensor_add(sa[:, :], sa[:, :], sb[:, :])
                    s = wp.tile([128, HIDDEN], BF16, tag="s")
                    nc.vector.tensor_add(s[:, :], sa[:, :], sc[:, :])
                    # transpose via PE
                    ps_nt = psp.tile([128, HIDDEN], F32, tag="ps_h0")
                    for b in range(4):
                        nc.tensor.matmul(
                            out=ps_nt[:, b * 128:(b + 1) * 128],
                            lhsT=s[:, b * 128:(b + 1) * 128],
                            rhs=ident[:, :],
                            start=True, stop=True,
                        )
                    nt = wp.tile([128, HIDDEN], BF16, tag="nt")
                    nc.scalar.copy(nt[:, :], ps_nt[:, :])
                    ps_o = psp.tile([128, HIDDEN], F32, tag="ps_b")
                    for b in range(4):
                        nc.tensor.matmul(
                            out=ps_o[:, :],
                            lhsT=nt[:, b * 128:(b + 1) * 128],
                            rhs=whs[:, b * HIDDEN:(b + 1) * HIDDEN],
                            start=(b == 0), stop=(b == 3),
                        )
                    tnew = op.tile([128, HIDDEN], F32, tag="tnew")
                    nc.vector.tensor_add(
                        tnew[:, :], ps_o[:, :], base_t[:, c * HIDDEN:(c + 1) * HIDDEN]
                    )
                    if last:
                        hf = op.tile([128, HIDDEN], F32, tag="hf")
                        nc.scalar.activation(
                            hf[:, :], tnew[:, :], mybir.ActivationFunctionType.Relu
                        )
                        nc.sync.dma_start(out=h_out[a0:a0 + m, :], in_=hf[:m, :])
                    else:
                        hb = op.tile([128, HIDDEN], FP8, tag="hb")
                        nc.scalar.activation(
                            hb[:, :], tnew[:, :], mybir.ActivationFunctionType.Relu
                        )
                        nc.sync.dma_start(
                            out=agin8[a0:a0 + m, :], in_=hb[:m, :].bitcast(U16))
                        cc = emit_ags(nc, agin8, tables[d + 1], c) or cc


    nc.finalize()
    return nc


def _prepare_inputs(fatoms, fbonds, W_nin, W_node, aoutgraph, in_n):
    fatoms = np.asarray(fatoms, dtype=np.float32)
    fbonds = np.asarray(fbonds, dtype=np.float32)
    W_nin = np.asarray(W_nin, dtype=np.float32)
    W_node = np.asarray(W_node, dtype=np.float32)
    aout = np.asarray(aoutgraph, dtype=np.int64)
    in_n = np.asarray(in_n, dtype=np.int64)

    # atom id -> table row (segment-major two-bank layout)
    perm = np.empty(N_ATOMS, dtype=np.int64)
    rows_core0 = np.array([_row_of(0, a) for a in range(A_LOC)], dtype=np.int64)
    seg_of = np.empty(A_LOC, dtype=np.int64)
    seg_m = np.empty(A_LOC, dtype=np.int64)
    for i, (a0, mseg) in enumerate(SEGS):
        seg_of[a0:a0 + mseg] = i
        seg_m[a0:a0 + mseg] = mseg
    for k in range(NCORES):
        perm[k * A_LOC:(k + 1) * A_LOC] = rows_core0 + k * seg_m
    # src[a,j]: table row of source atom, or -1 for the padding bond
    src_atom = np.where(aout > 0, in_n[np.maximum(aout - 1, 0)], -1)
    src_row = np.where(src_atom >= 0, perm[np.maximum(src_atom, 0)], -1)

    w_nin_t = np.ascontiguousarray(W_nin.T).astype(ml_dtypes.bfloat16)
    w_h_t = np.ascontiguousarray(W_node[:, :HIDDEN].T).astype(ml_dtypes.bfloat16)
    wb = W_node[:, HIDDEN:]
    wb_rep = np.ascontiguousarray(np.tile(wb.T, (MAX_NB, 1))).astype(ml_dtypes.bfloat16)

    in_maps = []
    for k in range(NCORES):
        sh = slice(k * A_LOC, (k + 1) * A_LOC)
        fat = np.ascontiguousarray(fatoms[sh].T).astype(ml_dtypes.bfloat16)
        fbg = fbonds[aout[sh]].reshape(A_LOC, FB)
        fbg_t = np.ascontiguousarray(fbg.T).astype(ml_dtypes.bfloat16)
        rows_k = np.where(src_row[sh] >= 0, src_row[sh], ZROW)  # [6250, 6]
        rows_k = np.sort(rows_k, axis=1)  # ascending: call j only needs rows < L[c,j]
        arr = np.full((128, NCHUNK * MAX_NB), ZROW, dtype=np.int32)
        for c in range(NCHUNK):
            mm = 128 if c < NCHUNK - 1 else LAST_M
            arr[:mm, c * MAX_NB:(c + 1) * MAX_NB] = rows_k[c * 128:c * 128 + mm]
        in_maps.append({
            "fatoms_t": fat,
            "fbg_t": fbg_t,
            "src_idx": arr,
            "w_nin_t": w_nin_t,
            "wb_rep": wb_rep,
            "w_h_t": w_h_t,
        })
    # static per-(chunk, call) gather row limits: max over cores
    L = np.zeros((NCHUNK, MAX_NB), dtype=np.int64)
    for im in in_maps:
        a = im["src_idx"]
        for c in range(NCHUNK):
            L[c] = np.maximum(L[c], a[:, c * MAX_NB:(c + 1) * MAX_NB].max(axis=0) + 1)
    return in_maps, L


_cached_nc = None
_cached_key = None


def _get_nc(L):
    global _cached_nc, _cached_key
    key = L.tobytes()
    if _cached_nc is None or _cached_key != key:
        _cached_nc = build_nc(L)
        _cached_key = key
    return _cached_nc


def run(inputs, trace=False):
    in_maps, L = _prepare_inputs(**inputs)
    nc = _get_nc(L)
    res = run_bass_kernel_spmd(
        nc, in_maps, core_ids=list(range(NCORES)), trace=trace
    )
    h_full = np.concatenate([res.results[c]["h_out"] for c in range(NCORES)], axis=0)
    out = np.ascontiguousarray(h_full.T)
    return out, res


def kernel(**inputs) -> np.ndarray:
    trace = bool(int(os.environ.get("TRN_KERNEL_TRACE", "0")))
    out, _ = run(inputs, trace=trace)
    return out
